# revision 3
# baseline (speedup 1.0000x reference)
"""Trainium2 Bass kernel for MultiHeadSelfAttention with ALiBi + adj bias.

Reference computation (B=2, L=2048, H=1024, NH=16, HS=64):
    xp = x @ weights + in_bias                  # [b, l, 3h], per-head interleaved qkv
    q, k, v per head; att = q k^T / 8 + alibi + gamma*adj; softmax
    out = (att @ v) @ out_w + out_bias

Sharding: 8 cores = 2 batches x 4 head-groups. Group g handles heads
{2g, 2g+1, 8+2g, 9+2g} (2 ALiBi heads + 2 bias-free heads per core for
load balance; heads 8..15 have zero ALiBi slope).

Per-core device strategy (all matmuls fp32r unless noted; PE-centric):
  - Host passes x^T, so QKV projection produces Q^T/K^T in head-column-major
    [cols, tokens] (in_bias folded via K=1 outer-product matmul into PSUM;
    1/8 attention scale folded into the Q evacuation on ScalarE)
    and V token-major [tokens, vcols] with an appended ones-column
    (gives softmax denominators for free during att@V).
  - Attention computed transposed: S^T[j,i] = K^T.T @ Q^T. ALiBi dist and
    gamma*adj biases are accumulated straight into PSUM with scaled-identity
    bf16 matmuls (PE adds at 307G elem/s vs 123G on DVE).
  - Single ScalarE pass: E^T = exp(PSUM) -> bf16 SBUF. No row-max pass
    (logits bounded ~<10, exp cannot overflow in fp32).
  - att@V: outT[d,i] += V_aug^T E^T, row 64 = denominators. Normalize via
    VectorE reciprocal + PE ones-outer-product broadcast + one small TT mult.
  - Output projection emits y^T partial sums [1024, 2048]; host sums the 4
    cores of each batch, transposes, and adds out_bias.
"""

import numpy as np
import ml_dtypes
from contextlib import ExitStack

import concourse.tile as tile
from concourse import bacc, mybir
from concourse import bass_utils

F32 = mybir.dt.float32
F32R = mybir.dt.float32r
BF16 = mybir.dt.bfloat16
AF = mybir.ActivationFunctionType

B, L, H, NH = 2, 2048, 1024, 16
HS = 64
NHL = 4            # local heads per core
P = 128            # partition tile
IC = 512           # i-chunk width (one PSUM bank of fp32)
NI = L // IC       # 4 i-chunks
NJ = L // P        # 16 j tiles
KT = H // P        # 8 contraction tiles over hidden
SCALE = 0.125      # 1/sqrt(HS)

# extra run kwargs injected by test harness (e.g. trace=True)
RUN_KWARGS: dict = {}

_cache: dict = {}


def _build_program():
    nc = bacc.Bacc("TRN2", target_bir_lowering=False, debug=False,
                   enable_asserts=False, num_devices=8)

    xT = nc.dram_tensor("xT", [H, L], F32R, kind="ExternalInput").ap()
    wqk = nc.dram_tensor("wqk", [H, 8 * HS], F32R, kind="ExternalInput").ap()
    wv = nc.dram_tensor("wv", [H, NHL * 65], F32R, kind="ExternalInput").ap()
    qkb = nc.dram_tensor("qkb", [1, 8 * HS], F32R, kind="ExternalInput").ap()
    vb = nc.dram_tensor("vb", [1, NHL * 65], F32R, kind="ExternalInput").ap()
    ones = nc.dram_tensor("ones", [1, IC], F32R, kind="ExternalInput").ap()
    adjT = nc.dram_tensor("adjT", [L, L], BF16, kind="ExternalInput").ap()
    dist = nc.dram_tensor("dist", [L, L], BF16, kind="ExternalInput").ap()
    gI = nc.dram_tensor("gI", [P, NHL * P], BF16, kind="ExternalInput").ap()
    dI = nc.dram_tensor("dI", [P, 2 * P], BF16, kind="ExternalInput").ap()
    ow = nc.dram_tensor("ow", [NHL * HS, H], F32R, kind="ExternalInput").ap()
    yT = nc.dram_tensor("yT", [H, L], F32, kind="ExternalOutput").ap()

    with tile.TileContext(nc) as tc, ExitStack() as ctx:
        # ---- long-lived tensors ----
        persist = ctx.enter_context(tc.tile_pool(name="persist", bufs=1))
        # Q^T/K^T col-major: 4 tiles [128, L]; tiles 0-1 = q of local heads
        # (0,1) and (2,3); tiles 2-3 = k likewise. 1/8 scale folded into q.
        qk_sb = [persist.tile([P, L], F32R, tag=f"qk{m}", name=f"qk{m}") for m in range(4)]
        # V token-major bf16 with ones-columns: 16 tiles [128, 260]
        v_sb = [persist.tile([P, NHL * 65], BF16, tag=f"v{t}", name=f"v{t}") for t in range(NJ)]
        # normalized att^T, fp32r: 2 tiles [128, L] (4 heads x 64 rows)
        attn_sb = [persist.tile([P, L], F32R, tag=f"attn{m}", name=f"attn{m}") for m in range(2)]
        ones_sb = persist.tile([1, IC], F32R, tag="ones")
        nc.sync.dma_start(ones_sb[:], ones)
        gI_sb = persist.tile([P, NHL * P], BF16, tag="gI")
        nc.sync.dma_start(gI_sb[:], gI)
        dI_sb = persist.tile([P, 2 * P], BF16, tag="dI")
        nc.sync.dma_start(dI_sb[:], dI)
        qkb_sb = persist.tile([1, 8 * HS], F32R, tag="qkb")
        nc.sync.dma_start(qkb_sb[:], qkb)
        vb_sb = persist.tile([1, NHL * 65], F32R, tag="vb")
        nc.sync.dma_start(vb_sb[:], vb)

        # ================= Phase A: QKV projection =================
        with tc.tile_pool(name="xw", bufs=1) as xw_pool, \
             tc.tile_pool(name="psA", bufs=4, space="PSUM") as psA:
            xT_sb = [xw_pool.tile([P, L], F32R, tag=f"x{k}", name=f"xt{k}") for k in range(KT)]
            for k in range(KT):
                nc.sync.dma_start(xT_sb[k][:], xT[k * P:(k + 1) * P, :])
            wqk_sb = [xw_pool.tile([P, 8 * HS], F32R, tag=f"wqk{k}", name=f"wqk{k}")
                      for k in range(KT)]
            for k in range(KT):
                nc.sync.dma_start(wqk_sb[k][:], wqk[k * P:(k + 1) * P, :])
            wv_sb = [xw_pool.tile([P, NHL * 65], F32R, tag=f"wv{k}", name=f"wv{k}")
                     for k in range(KT)]
            for k in range(KT):
                nc.sync.dma_start(wv_sb[k][:], wv[k * P:(k + 1) * P, :])

            # Q^T / K^T: psum[col-tile m, token-chunk n]
            for m in range(4):
                for n in range(NI):
                    ps = psA.tile([P, IC], F32, tag="qkp")
                    nc.tensor.matmul(ps[:], qkb_sb[:, m * P:(m + 1) * P],
                                     ones_sb[:], start=True, stop=False)
                    for k in range(KT):
                        nc.tensor.matmul(
                            ps[:], wqk_sb[k][:, m * P:(m + 1) * P],
                            xT_sb[k][:, n * IC:(n + 1) * IC],
                            start=False, stop=(k == KT - 1))
                    nc.scalar.activation(
                        qk_sb[m][:, n * IC:(n + 1) * IC], ps[:], AF.Copy,
                        scale=SCALE if m < 2 else 1.0)

            # V token-major (+bias +ones col via K=1 outer product)
            for t in range(NJ):
                ps = psA.tile([P, NHL * 65], F32, tag="vp")
                nc.tensor.matmul(ps[:], ones_sb[:, 0:P], vb_sb[:],
                                 start=True, stop=False)
                for k in range(KT):
                    nc.tensor.matmul(ps[:], xT_sb[k][:, t * P:(t + 1) * P],
                                     wv_sb[k][:], start=False,
                                     stop=(k == KT - 1))
                nc.scalar.activation(v_sb[t][:], ps[:], AF.Copy)

        # ================= Phase B: attention =================
        with tc.tile_pool(name="bias_chunks", bufs=2) as bc_pool, \
             tc.tile_pool(name="e_pool", bufs=6) as e_pool, \
             tc.tile_pool(name="r_pool", bufs=4) as r_pool, \
             tc.tile_pool(name="psS", bufs=4, space="PSUM") as psS, \
             tc.tile_pool(name="psO", bufs=2, space="PSUM") as psO, \
             tc.tile_pool(name="psR", bufs=2, space="PSUM") as psR:
            for i in range(NI):
                isl = slice(i * IC, (i + 1) * IC)
                adj_ch = bc_pool.tile([P, NJ * IC], BF16, tag="adj")
                for j in range(NJ):
                    nc.sync.dma_start(adj_ch[:, j * IC:(j + 1) * IC],
                                      adjT[j * P:(j + 1) * P, isl])
                dist_ch = bc_pool.tile([P, NJ * IC], BF16, tag="dist")
                for j in range(NJ):
                    nc.sync.dma_start(dist_ch[:, j * IC:(j + 1) * IC],
                                      dist[j * P:(j + 1) * P, isl])
                for h in range(NHL):
                    hp = slice((h % 2) * HS, (h % 2) * HS + HS)
                    alibi = h < 2
                    avp = psO.tile([65, IC], F32, tag="avp")
                    for j in range(NJ):
                        sp = psS.tile([P, IC], F32, tag="sp")
                        nc.tensor.matmul(
                            sp[:], qk_sb[2 + h // 2][hp, j * P:(j + 1) * P],
                            qk_sb[h // 2][hp, isl], start=True, stop=False)
                        nc.tensor.matmul(
                            sp[:], gI_sb[:, h * P:(h + 1) * P],
                            adj_ch[:, j * IC:(j + 1) * IC],
                            start=False, stop=not alibi)
                        if alibi:
                            nc.tensor.matmul(
                                sp[:], dI_sb[:, h * P:(h + 1) * P],
                                dist_ch[:, j * IC:(j + 1) * IC],
                                start=False, stop=True)
                        et = e_pool.tile([P, IC], BF16, tag="et")
                        nc.scalar.activation(et[:], sp[:], AF.Exp)
                        nc.tensor.matmul(avp[:],
                                         v_sb[j][:, h * 65:(h + 1) * 65],
                                         et[:], start=(j == 0),
                                         stop=(j == NJ - 1))
                    # normalize: values/denominator -> attn_sb
                    rt = r_pool.tile([1, IC], F32R, tag="rt")
                    with nc.allow_low_precision(reason="f32r denom reciprocal"):
                        nc.vector.reciprocal(rt[:], avp[64:65, :])
                    rbp = psR.tile([HS, IC], F32, tag="rbp")
                    nc.tensor.matmul(rbp[:], ones_sb[:, 0:HS], rt[:],
                                     start=True, stop=True)
                    rbs = r_pool.tile([HS, IC], F32, tag="rbs")
                    nc.scalar.activation(rbs[:], rbp[:], AF.Copy)
                    nc.vector.tensor_mul(
                        attn_sb[h // 2][hp.start:hp.start + HS, isl],
                        avp[0:HS, :], rbs[:])

        # ================= Phase C: output projection =================
        with tc.tile_pool(name="ow_pool", bufs=1) as ow_pool, \
             tc.tile_pool(name="y_pool", bufs=3) as y_pool, \
             tc.tile_pool(name="psY", bufs=4, space="PSUM") as psY:
            ow_sb = [ow_pool.tile([P, H], F32R, tag=f"ow{k}", name=f"ow{k}") for k in range(2)]
            for k in range(2):
                nc.sync.dma_start(ow_sb[k][:], ow[k * P:(k + 1) * P, :])
            for m in range(H // P):
                yt = y_pool.tile([P, L], F32, tag="yt")
                for n in range(NI):
                    ps = psY.tile([P, IC], F32, tag="yp")
                    for k in range(2):
                        nc.tensor.matmul(
                            ps[:], ow_sb[k][:, m * P:(m + 1) * P],
                            attn_sb[k][:, n * IC:(n + 1) * IC],
                            start=(k == 0), stop=(k == 1))
                    nc.scalar.activation(yt[:, n * IC:(n + 1) * IC], ps[:],
                                         AF.Copy)
                nc.sync.dma_start(yT[m * P:(m + 1) * P, :], yt[:])

    nc.compile()
    return nc


def _alibi_slopes():
    n = NH // 2
    start = 2.0 ** (-(2.0 ** (-(np.log2(n) - 3.0))))
    s = np.array([start * start ** i for i in range(n)], dtype=np.float32)
    return np.concatenate([s, np.zeros(n, dtype=np.float32)])


def _build_in_maps(x, adj, weights, in_bias, gamma, out_w):
    slopes = _alibi_slopes()
    bf = ml_dtypes.bfloat16
    ar = np.arange(L, dtype=np.float32)
    dist = -np.abs(ar[None, :] - ar[:, None]).astype(bf)
    ones = np.ones((1, IC), dtype=np.float32)
    eye = np.eye(P, dtype=np.float32)

    in_maps = []
    for core in range(8):
        b, g = divmod(core, 4)
        heads = [2 * g, 2 * g + 1, 8 + 2 * g, 9 + 2 * g]
        xTb = np.ascontiguousarray(x[b].T)
        adjTb = np.ascontiguousarray(adj[b, 0].T).astype(bf)

        qcols = np.concatenate([np.arange(192 * h, 192 * h + 64)
                                for h in heads])
        kcols = qcols + 64
        vcols = qcols + 128
        wqk = np.ascontiguousarray(weights[:, np.concatenate([qcols, kcols])])
        qkb = in_bias[0, 0, np.concatenate([qcols, kcols])].reshape(1, -1)
        wv = np.zeros((H, NHL * 65), dtype=np.float32)
        vbr = np.zeros((1, NHL * 65), dtype=np.float32)
        for hl in range(NHL):
            wv[:, 65 * hl:65 * hl + 64] = weights[:, vcols[64 * hl:64 * hl + 64]]
            vbr[0, 65 * hl:65 * hl + 64] = in_bias[0, 0, vcols[64 * hl:64 * hl + 64]]
            vbr[0, 65 * hl + 64] = 1.0
        gIm = np.concatenate(
            [gamma[0, h, 0, 0] * eye for h in heads], axis=1).astype(bf)
        dIm = np.concatenate(
            [slopes[h] * eye for h in heads[:2]], axis=1).astype(bf)
        owm = np.ascontiguousarray(
            out_w[np.concatenate([np.arange(64 * h, 64 * h + 64)
                                  for h in heads]), :])
        in_maps.append({
            "xT": xTb, "wqk": wqk, "wv": wv,
            "qkb": np.ascontiguousarray(qkb, dtype=np.float32),
            "vb": vbr, "ones": ones, "adjT": adjTb, "dist": dist,
            "gI": np.ascontiguousarray(gIm), "dI": np.ascontiguousarray(dIm),
            "ow": owm,
        })
    return in_maps


def kernel(x, adj, weights, in_bias, out_w, out_bias, gamma):
    x = np.asarray(x, dtype=np.float32)
    adj = np.asarray(adj, dtype=np.float32)
    weights = np.asarray(weights, dtype=np.float32)
    in_bias = np.asarray(in_bias, dtype=np.float32)
    out_w = np.asarray(out_w, dtype=np.float32)
    out_bias = np.asarray(out_bias, dtype=np.float32)
    gamma = np.asarray(gamma, dtype=np.float32)

    if "nc" not in _cache:
        _cache["nc"] = _build_program()
    nc = _cache["nc"]

    in_maps = _build_in_maps(x, adj, weights, in_bias, gamma, out_w)
    res = bass_utils.run_bass_kernel_spmd(nc, in_maps, core_ids=list(range(8)),
                                          **RUN_KWARGS)
    _cache["last_result"] = res

    out = np.empty((B, L, H), dtype=np.float32)
    for b in range(B):
        acc = res.results[4 * b]["yT"].astype(np.float64)
        for g in range(1, 4):
            acc += res.results[4 * b + g]["yT"]
        out[b] = acc.T + out_bias[0, 0][None, :]
    return out


# revision 18
# speedup vs baseline: 483.4976x; 483.4976x over previous
"""Trainium2 Bass kernel for MultiHeadSelfAttention with ALiBi + adj bias.

Reference computation (B=2, L=2048, H=1024, NH=16, HS=64):
    xp = x @ weights + in_bias                  # [b, l, 3h], per-head interleaved qkv
    q, k, v per head; att = q k^T / 8 + alibi + gamma*adj; softmax
    out = (att @ v) @ out_w + out_bias

Sharding: 8 cores = 2 batches x 4 head-groups. Group g handles heads
{2g, 2g+1, 8+2g, 9+2g} (2 ALiBi heads + 2 bias-free heads per core for
load balance; heads 8..15 have zero ALiBi slope).

Per-core device strategy (all matmuls fp32r unless noted; PE-centric):
  - Host passes x^T, so QKV projection produces Q^T/K^T in head-column-major
    [cols, tokens] (in_bias folded via K=1 outer-product matmul into PSUM;
    1/8 attention scale folded into the Q evacuation on ScalarE)
    and V token-major [tokens, vcols] with an appended ones-column
    (gives softmax denominators for free during att@V).
  - Attention computed transposed: S^T[j,i] = K^T.T @ Q^T. ALiBi dist and
    gamma*adj biases are accumulated straight into PSUM with scaled-identity
    bf16 matmuls (PE adds at 307G elem/s vs 123G on DVE).
  - Single ScalarE pass: E^T = exp(PSUM) -> bf16 SBUF, 1024 wide to amortize
    the per-op overhead. No row-max pass (logits bounded ~<10, exp cannot
    overflow in fp32).
  - att@V: outT[d,i] += V_aug^T E^T, row 64 = denominators. Normalize via
    VectorE reciprocal + PE ones-outer-product broadcast + one small TT mult.
  - Output projection emits y^T partial sums [1024, 2048]; host sums the 4
    cores of each batch, transposes, and adds out_bias.
"""

import numpy as np
import ml_dtypes
from contextlib import ExitStack

import concourse.tile as tile
from concourse import bacc, mybir
from concourse import bass_utils

F32 = mybir.dt.float32
F32R = mybir.dt.float32r
BF16 = mybir.dt.bfloat16
F16 = mybir.dt.float16
AF = mybir.ActivationFunctionType

B, L, H, NH = 2, 2048, 1024, 16
HS = 64
NHL = 4            # local heads per core
P = 128            # partition tile
IC = 1024          # i-chunk width (two PSUM banks of fp32)
NI = L // IC       # 2 i-chunks
NJ = L // P        # 16 j tiles
KT = H // P        # 8 contraction tiles over hidden
SCALE = 0.125      # 1/sqrt(HS)
MMN = 512          # fp32r moving-operand cap

# extra run kwargs injected by test harness (e.g. trace=True)
RUN_KWARGS: dict = {}

_cache: dict = {}


def _build_program(with_qk_bias=True):
    nc = bacc.Bacc("TRN2", target_bir_lowering=False, debug=False,
                   enable_asserts=False, num_devices=8)

    xT = nc.dram_tensor("xT", [H, L], F32R, kind="ExternalInput").ap()
    wqk = nc.dram_tensor("wqk", [H, 8 * HS], F32R, kind="ExternalInput").ap()
    wv = nc.dram_tensor("wv", [H, NHL * 65], F32R, kind="ExternalInput").ap()
    qkb = (nc.dram_tensor("qkb", [1, 8 * HS], F32R, kind="ExternalInput").ap()
           if with_qk_bias else None)
    vb = nc.dram_tensor("vb", [1, NHL * 65], F32R, kind="ExternalInput").ap()
    ones = nc.dram_tensor("ones", [1, MMN], F32R, kind="ExternalInput").ap()
    cb = nc.dram_tensor("cb", [L, NHL * L], F16, kind="ExternalInput").ap()
    ident = nc.dram_tensor("ident", [P, P], F16, kind="ExternalInput").ap()
    ow = nc.dram_tensor("ow", [NHL * HS, H], F32R, kind="ExternalInput").ap()
    yT = nc.dram_tensor("yT", [H, L], F32, kind="ExternalOutput").ap()

    with tile.TileContext(nc) as tc, ExitStack() as ctx:
        # ---- long-lived tensors ----
        persist = ctx.enter_context(tc.tile_pool(name="persist", bufs=1))
        # Q^T/K^T col-major: 4 tiles [128, L]; tiles 0-1 = q of local heads
        # (0,1) and (2,3); tiles 2-3 = k likewise. 1/8 scale folded into q.
        qk_sb = [persist.tile([P, L], F32R, tag=f"qk{m}", name=f"qk{m}")
                 for m in range(4)]
        # V token-major bf16 with ones-columns: 16 tiles [128, 260]
        v_sb = [persist.tile([P, NHL * 65], BF16, tag=f"v{t}", name=f"v{t}")
                for t in range(NJ)]
        # normalized att^T, fp32r: 2 tiles [128, L] (4 heads x 64 rows)
        attn_sb = [persist.tile([P, L], F32R, tag=f"attn{m}", name=f"attn{m}")
                   for m in range(2)]
        ones_sb = persist.tile([1, MMN], F32R, tag="ones")
        nc.sync.dma_start(ones_sb[:], ones)
        id_sb = persist.tile([P, P], F16, tag="ident")
        nc.sync.dma_start(id_sb[:], ident)
        if with_qk_bias:
            qkb_sb = persist.tile([1, 8 * HS], F32R, tag="qkb")
            nc.sync.dma_start(qkb_sb[:], qkb)
        vb_sb = persist.tile([1, NHL * 65], F32R, tag="vb")
        nc.sync.dma_start(vb_sb[:], vb)

        # ================= Phase A: QKV projection =================
        with tc.tile_pool(name="xw", bufs=1) as xw_pool, \
             tc.tile_pool(name="psA", bufs=2, space="PSUM") as psA:
            wv_sb = [xw_pool.tile([P, NHL * 65], F32R, tag=f"wv{k}",
                                  name=f"wv{k}") for k in range(KT)]
            for k in range(KT):
                nc.sync.dma_start(wv_sb[k][:], wv[k * P:(k + 1) * P, :])
            # xT chunked [128, 512] chunk-major so early chunks of every
            # k-tile land first and the PE can start while DMA continues
            xT_sb = [xw_pool.tile([P, L], F32R, tag=f"x{k}", name=f"xt{k}")
                     for k in range(KT)]
            wqk_sb = [xw_pool.tile([P, 8 * HS], F32R, tag=f"wqk{k}",
                                   name=f"wqk{k}") for k in range(KT)]
            for k in range(KT):
                nc.sync.dma_start(xT_sb[k][:, 0:MMN],
                                  xT[k * P:(k + 1) * P, 0:MMN])
            for k in range(KT):
                nc.sync.dma_start(wqk_sb[k][:], wqk[k * P:(k + 1) * P, :])
            for c in range(1, L // MMN):
                for k in range(KT):
                    nc.sync.dma_start(xT_sb[k][:, c * MMN:(c + 1) * MMN],
                                      xT[k * P:(k + 1) * P,
                                         c * MMN:(c + 1) * MMN])

            # V token-major first: needs only thin xT slices, covers the
            # rest of the xT DMA
            for t in range(NJ):
                ps = psA.tile([P, NHL * 65], F32, tag="vp", bufs=3)
                nc.tensor.matmul(ps[:], ones_sb[:, 0:P], vb_sb[:],
                                 start=True, stop=False)
                for k in range(KT):
                    nc.tensor.matmul(ps[:], xT_sb[k][:, t * P:(t + 1) * P],
                                     wv_sb[k][:], start=False,
                                     stop=(k == KT - 1))
                nc.scalar.activation(v_sb[t][:], ps[:], AF.Copy)

            # Q^T / K^T: psum[col-tile m, token-chunk n of 1024]
            for n in range(NI):
                for m in (1, 3, 0, 2):
                    ps = psA.tile([P, IC], F32, tag="qkp")
                    for half in range(2):
                        hs = slice(half * MMN, (half + 1) * MMN)
                        nsl = slice(n * IC + half * MMN,
                                    n * IC + (half + 1) * MMN)
                        if with_qk_bias:
                            nc.tensor.matmul(ps[:, hs],
                                             qkb_sb[:, m * P:(m + 1) * P],
                                             ones_sb[:], start=True,
                                             stop=False)
                        for k in range(KT):
                            nc.tensor.matmul(
                                ps[:, hs], wqk_sb[k][:, m * P:(m + 1) * P],
                                xT_sb[k][:, nsl],
                                start=(k == 0 and not with_qk_bias),
                                stop=(k == KT - 1))
                    nc.scalar.activation(
                        qk_sb[m][:, n * IC:(n + 1) * IC], ps[:], AF.Copy,
                        scale=SCALE if m < 2 else 1.0)

        # ================= Phase B: attention =================
        # head order: non-ALiBi heads (2,3) first so the dist chunk DMA for
        # the next i-chunk hides behind their compute.
        with tc.tile_pool(name="bias_chunks", bufs=2) as bc_pool, \
             tc.tile_pool(name="e_pool", bufs=6) as e_pool, \
             tc.tile_pool(name="r_pool", bufs=2) as r_pool, \
             tc.tile_pool(name="psS", bufs=3, space="PSUM") as psS, \
             tc.tile_pool(name="psO", bufs=1, space="PSUM") as psO:
            for i in range(NI):
                isl = slice(i * IC, (i + 1) * IC)
                for h in range(NHL):
                    hp = slice((h % 2) * HS, (h % 2) * HS + HS)
                    cb_ch = bc_pool.tile([P, NJ * IC], F16, tag="cb")
                    for j in range(NJ):
                        nc.sync.dma_start(
                            cb_ch[:, j * IC:(j + 1) * IC],
                            cb[j * P:(j + 1) * P,
                               h * L + i * IC:h * L + (i + 1) * IC])
                    avp = psO.tile([65, IC], F32, tag="avp")
                    for j in range(NJ):
                        sp = psS.tile([P, IC], F32, tag="sp")
                        for half in range(2):
                            nc.tensor.matmul(
                                sp[:, half * MMN:(half + 1) * MMN],
                                qk_sb[2 + h // 2][hp, j * P:(j + 1) * P],
                                qk_sb[h // 2][hp, i * IC + half * MMN:
                                              i * IC + (half + 1) * MMN],
                                start=True, stop=False)
                        for half in range(2):
                            hs = slice(half * MMN, (half + 1) * MMN)
                            nc.tensor.matmul(
                                sp[:, hs], id_sb[:],
                                cb_ch[:, j * IC + half * MMN:
                                      j * IC + (half + 1) * MMN],
                                start=False, stop=True)
                        et = e_pool.tile([P, IC], BF16, tag="et")
                        nc.scalar.activation(et[:], sp[:], AF.Exp)
                        for half in range(2):
                            hs = slice(half * MMN, (half + 1) * MMN)
                            nc.tensor.matmul(avp[:, hs],
                                             v_sb[j][:, h * 65:(h + 1) * 65],
                                             et[:, hs], start=(j == 0),
                                             stop=(j == NJ - 1))
                    # normalize: values/denominator -> attn_sb
                    rt = r_pool.tile([1, IC], F32R, tag="rt")
                    with nc.allow_low_precision(reason="f32r denom recip"):
                        nc.vector.reciprocal(rt[:], avp[64:65, :])
                    rbs = r_pool.tile([HS, IC], F32, tag="rbs")
                    nc.gpsimd.partition_broadcast(rbs[:], rt[:].bitcast(F32))
                    nc.vector.tensor_mul(
                        attn_sb[h // 2][hp.start:hp.start + HS, isl],
                        avp[0:HS, :], rbs[:])

        # ================= Phase C: output projection =================
        with tc.tile_pool(name="ow_pool", bufs=1) as ow_pool, \
             tc.tile_pool(name="y_pool", bufs=3) as y_pool, \
             tc.tile_pool(name="psY", bufs=3, space="PSUM") as psY:
            ow_sb = [ow_pool.tile([P, H], F32R, tag=f"ow{k}", name=f"ow{k}")
                     for k in range(2)]
            for k in range(2):
                nc.sync.dma_start(ow_sb[k][:], ow[k * P:(k + 1) * P, :])
            for m in range(H // P):
                yt = y_pool.tile([P, L], F32, tag="yt")
                for n in range(NI):
                    ps = psY.tile([P, IC], F32, tag="yp")
                    for half in range(2):
                        hs = slice(half * MMN, (half + 1) * MMN)
                        nsl = slice(n * IC + half * MMN,
                                    n * IC + (half + 1) * MMN)
                        for k in range(2):
                            nc.tensor.matmul(
                                ps[:, hs], ow_sb[k][:, m * P:(m + 1) * P],
                                attn_sb[k][:, nsl],
                                start=(k == 0), stop=(k == 1))
                    if (m + n) % 2 == 0:
                        nc.vector.tensor_copy(yt[:, n * IC:(n + 1) * IC],
                                              ps[:])
                    else:
                        nc.scalar.activation(yt[:, n * IC:(n + 1) * IC],
                                             ps[:], AF.Copy)
                    eng = nc.sync if n % 2 == 0 else nc.scalar
                    eng.dma_start(
                        yT[m * P:(m + 1) * P, n * IC:(n + 1) * IC],
                        yt[:, n * IC:(n + 1) * IC])

    nc.compile()
    return nc


def _alibi_slopes():
    n = NH // 2
    start = 2.0 ** (-(2.0 ** (-(np.log2(n) - 3.0))))
    s = np.array([start * start ** i for i in range(n)], dtype=np.float32)
    return np.concatenate([s, np.zeros(n, dtype=np.float32)])


def _build_in_maps(x, adj, weights, in_bias, gamma, out_w):
    from concurrent.futures import ThreadPoolExecutor
    slopes = _alibi_slopes()
    ar = np.arange(L, dtype=np.float32)
    dist = -np.abs(ar[None, :] - ar[:, None])
    ones = np.ones((1, MMN), dtype=np.float32)
    eye = np.eye(P, dtype=np.float32)
    ident = eye.astype(np.float16)
    adjT_by_b = [np.ascontiguousarray(adj[b, 0].T) for b in range(B)]
    xT_by_b = [np.ascontiguousarray(x[b].T) for b in range(B)]

    def _make_cb(core):
        b, g = divmod(core, 4)
        heads = [2 * g, 2 * g + 1, 8 + 2 * g, 9 + 2 * g]
        cbm = np.empty((L, NHL * L), dtype=np.float16)
        for hl, hh in enumerate(heads):
            t = gamma[0, hh, 0, 0] * adjT_by_b[b]
            if slopes[hh] != 0.0:
                t = t + slopes[hh] * dist
            cbm[:, hl * L:(hl + 1) * L] = t
        return cbm

    with ThreadPoolExecutor(max_workers=8) as ex:
        cb_by_core = list(ex.map(_make_cb, range(8)))

    in_maps = []
    for core in range(8):
        b, g = divmod(core, 4)
        heads = [2 * g, 2 * g + 1, 8 + 2 * g, 9 + 2 * g]
        xTb = xT_by_b[b]
        cbm = cb_by_core[core]

        qcols = np.concatenate([np.arange(192 * h, 192 * h + 64)
                                for h in heads])
        kcols = qcols + 64
        vcols = qcols + 128
        wqk = np.ascontiguousarray(weights[:, np.concatenate([qcols, kcols])])
        qkb = in_bias[0, 0, np.concatenate([qcols, kcols])].reshape(1, -1)
        wv = np.zeros((H, NHL * 65), dtype=np.float32)
        vbr = np.zeros((1, NHL * 65), dtype=np.float32)
        for hl in range(NHL):
            wv[:, 65 * hl:65 * hl + 64] = weights[:, vcols[64 * hl:64 * hl + 64]]
            vbr[0, 65 * hl:65 * hl + 64] = in_bias[0, 0, vcols[64 * hl:64 * hl + 64]]
            vbr[0, 65 * hl + 64] = 1.0
        owm = np.ascontiguousarray(
            out_w[np.concatenate([np.arange(64 * h, 64 * h + 64)
                                  for h in heads]), :])
        in_maps.append({
            "xT": xTb, "wqk": wqk, "wv": wv,
            "qkb": np.ascontiguousarray(qkb, dtype=np.float32),
            "vb": vbr, "ones": ones, "cb": cbm, "ident": ident,
            "ow": owm,
        })
    return in_maps


def kernel(x, adj, weights, in_bias, out_w, out_bias, gamma):
    x = np.asarray(x, dtype=np.float32)
    adj = np.asarray(adj, dtype=np.float32)
    weights = np.asarray(weights, dtype=np.float32)
    in_bias = np.asarray(in_bias, dtype=np.float32)
    out_w = np.asarray(out_w, dtype=np.float32)
    out_bias = np.asarray(out_bias, dtype=np.float32)
    gamma = np.asarray(gamma, dtype=np.float32)

    with_qk_bias = bool(np.any(in_bias[0, 0, :]))
    key = f"nc_{with_qk_bias}"
    if key not in _cache:
        _cache[key] = _build_program(with_qk_bias)
    nc = _cache[key]

    in_maps = _build_in_maps(x, adj, weights, in_bias, gamma, out_w)
    if not with_qk_bias:
        for m in in_maps:
            m.pop("qkb")
    res = bass_utils.run_bass_kernel_spmd(nc, in_maps, core_ids=list(range(8)),
                                          **RUN_KWARGS)
    _cache["last_result"] = res

    out = np.empty((B, L, H), dtype=np.float32)
    for b in range(B):
        acc = res.results[4 * b]["yT"].astype(np.float64)
        for g in range(1, 4):
            acc += res.results[4 * b + g]["yT"]
        out[b] = acc.T + out_bias[0, 0][None, :]
    return out


# revision 19
# speedup vs baseline: 483.7775x; 1.0006x over previous
"""Trainium2 Bass kernel for MultiHeadSelfAttention with ALiBi + adj bias.

Reference computation (B=2, L=2048, H=1024, NH=16, HS=64):
    xp = x @ weights + in_bias                  # [b, l, 3h], per-head interleaved qkv
    q, k, v per head; att = q k^T / 8 + alibi + gamma*adj; softmax
    out = (att @ v) @ out_w + out_bias

Sharding: 8 cores = 2 batches x 4 head-groups. Group g handles heads
{2g, 2g+1, 8+2g, 9+2g} (2 ALiBi heads + 2 bias-free heads per core for
load balance; heads 8..15 have zero ALiBi slope).

Per-core device strategy (all matmuls fp32r unless noted; PE-centric):
  - Host passes x^T, so QKV projection produces Q^T/K^T in head-column-major
    [cols, tokens] (in_bias folded via K=1 outer-product matmul into PSUM;
    1/8 attention scale folded into the Q evacuation on ScalarE)
    and V token-major [tokens, vcols] with an appended ones-column
    (gives softmax denominators for free during att@V).
  - Attention computed transposed: S^T[j,i] = K^T.T @ Q^T. ALiBi dist and
    gamma*adj biases are accumulated straight into PSUM with scaled-identity
    bf16 matmuls (PE adds at 307G elem/s vs 123G on DVE).
  - Single ScalarE pass: E^T = exp(PSUM) -> bf16 SBUF, 1024 wide to amortize
    the per-op overhead. No row-max pass (logits bounded ~<10, exp cannot
    overflow in fp32).
  - att@V: outT[d,i] += V_aug^T E^T, row 64 = denominators. Normalize via
    VectorE reciprocal + PE ones-outer-product broadcast + one small TT mult.
  - Output projection emits y^T partial sums [1024, 2048]; host sums the 4
    cores of each batch, transposes, and adds out_bias.
"""

import numpy as np
import ml_dtypes
from contextlib import ExitStack

import concourse.tile as tile
from concourse import bacc, mybir
from concourse import bass_utils

F32 = mybir.dt.float32
F32R = mybir.dt.float32r
BF16 = mybir.dt.bfloat16
F16 = mybir.dt.float16
AF = mybir.ActivationFunctionType

B, L, H, NH = 2, 2048, 1024, 16
HS = 64
NHL = 4            # local heads per core
P = 128            # partition tile
IC = 1024          # i-chunk width (two PSUM banks of fp32)
NI = L // IC       # 2 i-chunks
NJ = L // P        # 16 j tiles
KT = H // P        # 8 contraction tiles over hidden
SCALE = 0.125      # 1/sqrt(HS)
MMN = 512          # fp32r moving-operand cap

# extra run kwargs injected by test harness (e.g. trace=True)
RUN_KWARGS: dict = {}

_cache: dict = {}


def _build_program(with_qk_bias=True):
    nc = bacc.Bacc("TRN2", target_bir_lowering=False, debug=False,
                   enable_asserts=False, num_devices=8)

    xT = nc.dram_tensor("xT", [H, L], F32R, kind="ExternalInput").ap()
    wqk = nc.dram_tensor("wqk", [H, 8 * HS], F32R, kind="ExternalInput").ap()
    wv = nc.dram_tensor("wv", [H, NHL * 65], F32R, kind="ExternalInput").ap()
    qkb = (nc.dram_tensor("qkb", [1, 8 * HS], F32R, kind="ExternalInput").ap()
           if with_qk_bias else None)
    vb = nc.dram_tensor("vb", [1, NHL * 65], F32R, kind="ExternalInput").ap()
    ones = nc.dram_tensor("ones", [1, MMN], F32R, kind="ExternalInput").ap()
    cb = nc.dram_tensor("cb", [L, NHL * L], F16, kind="ExternalInput").ap()
    ident = nc.dram_tensor("ident", [P, P], F16, kind="ExternalInput").ap()
    ow = nc.dram_tensor("ow", [NHL * HS, H], F32R, kind="ExternalInput").ap()
    yT = nc.dram_tensor("yT", [H, L], F32, kind="ExternalOutput").ap()

    with tile.TileContext(nc) as tc, ExitStack() as ctx:
        # ---- long-lived tensors ----
        persist = ctx.enter_context(tc.tile_pool(name="persist", bufs=1))
        # Q^T/K^T col-major: 4 tiles [128, L]; tiles 0-1 = q of local heads
        # (0,1) and (2,3); tiles 2-3 = k likewise. 1/8 scale folded into q.
        qk_sb = [persist.tile([P, L], F32R, tag=f"qk{m}", name=f"qk{m}")
                 for m in range(4)]
        # V token-major bf16 with ones-columns: 16 tiles [128, 260]
        v_sb = [persist.tile([P, NHL * 65], BF16, tag=f"v{t}", name=f"v{t}")
                for t in range(NJ)]
        # normalized att^T, fp32r: 2 tiles [128, L] (4 heads x 64 rows)
        attn_sb = [persist.tile([P, L], F32R, tag=f"attn{m}", name=f"attn{m}")
                   for m in range(2)]
        ones_sb = persist.tile([1, MMN], F32R, tag="ones")
        nc.sync.dma_start(ones_sb[:], ones)
        id_sb = persist.tile([P, P], F16, tag="ident")
        nc.sync.dma_start(id_sb[:], ident)
        if with_qk_bias:
            qkb_sb = persist.tile([1, 8 * HS], F32R, tag="qkb")
            nc.sync.dma_start(qkb_sb[:], qkb)
        vb_sb = persist.tile([1, NHL * 65], F32R, tag="vb")
        nc.sync.dma_start(vb_sb[:], vb)

        # ================= Phase A: QKV projection =================
        with tc.tile_pool(name="xw", bufs=1) as xw_pool, \
             tc.tile_pool(name="psA", bufs=2, space="PSUM") as psA:
            wv_sb = [xw_pool.tile([P, NHL * 65], F32R, tag=f"wv{k}",
                                  name=f"wv{k}") for k in range(KT)]
            for k in range(KT):
                nc.sync.dma_start(wv_sb[k][:], wv[k * P:(k + 1) * P, :])
            # xT chunked [128, 512] chunk-major so early chunks of every
            # k-tile land first and the PE can start while DMA continues
            xT_sb = [xw_pool.tile([P, L], F32R, tag=f"x{k}", name=f"xt{k}")
                     for k in range(KT)]
            wqk_sb = [xw_pool.tile([P, 8 * HS], F32R, tag=f"wqk{k}",
                                   name=f"wqk{k}") for k in range(KT)]
            for k in range(KT):
                nc.sync.dma_start(xT_sb[k][:, 0:MMN],
                                  xT[k * P:(k + 1) * P, 0:MMN])
            for k in range(KT):
                nc.sync.dma_start(wqk_sb[k][:], wqk[k * P:(k + 1) * P, :])
            for c in range(1, L // MMN):
                for k in range(KT):
                    nc.sync.dma_start(xT_sb[k][:, c * MMN:(c + 1) * MMN],
                                      xT[k * P:(k + 1) * P,
                                         c * MMN:(c + 1) * MMN])

            # V token-major first: needs only thin xT slices, covers the
            # rest of the xT DMA
            for t in range(NJ):
                ps = psA.tile([P, NHL * 65], F32, tag="vp", bufs=3)
                nc.tensor.matmul(ps[:], ones_sb[:, 0:P], vb_sb[:],
                                 start=True, stop=False)
                for k in range(KT):
                    nc.tensor.matmul(ps[:], xT_sb[k][:, t * P:(t + 1) * P],
                                     wv_sb[k][:], start=False,
                                     stop=(k == KT - 1))
                nc.scalar.activation(v_sb[t][:], ps[:], AF.Copy)

            # Q^T / K^T: psum[col-tile m, token-chunk n of 1024]
            for n in range(NI):
                for m in (1, 3, 0, 2):
                    ps = psA.tile([P, IC], F32, tag="qkp")
                    for half in range(2):
                        hs = slice(half * MMN, (half + 1) * MMN)
                        nsl = slice(n * IC + half * MMN,
                                    n * IC + (half + 1) * MMN)
                        if with_qk_bias:
                            nc.tensor.matmul(ps[:, hs],
                                             qkb_sb[:, m * P:(m + 1) * P],
                                             ones_sb[:], start=True,
                                             stop=False)
                        for k in range(KT):
                            nc.tensor.matmul(
                                ps[:, hs], wqk_sb[k][:, m * P:(m + 1) * P],
                                xT_sb[k][:, nsl],
                                start=(k == 0 and not with_qk_bias),
                                stop=(k == KT - 1))
                    nc.scalar.activation(
                        qk_sb[m][:, n * IC:(n + 1) * IC], ps[:], AF.Copy,
                        scale=SCALE if m < 2 else 1.0)

        # ================= Phase B: attention =================
        # head order: non-ALiBi heads (2,3) first so the dist chunk DMA for
        # the next i-chunk hides behind their compute.
        with tc.tile_pool(name="bias_chunks", bufs=3) as bc_pool, \
             tc.tile_pool(name="e_pool", bufs=6) as e_pool, \
             tc.tile_pool(name="r_pool", bufs=2) as r_pool, \
             tc.tile_pool(name="psS", bufs=3, space="PSUM") as psS, \
             tc.tile_pool(name="psO", bufs=1, space="PSUM") as psO:
            for i in range(NI):
                isl = slice(i * IC, (i + 1) * IC)
                for h in range(NHL):
                    hp = slice((h % 2) * HS, (h % 2) * HS + HS)
                    cb_ch = bc_pool.tile([P, NJ * IC], F16, tag="cb")
                    for j in range(NJ):
                        nc.sync.dma_start(
                            cb_ch[:, j * IC:(j + 1) * IC],
                            cb[j * P:(j + 1) * P,
                               h * L + i * IC:h * L + (i + 1) * IC])
                    avp = psO.tile([65, IC], F32, tag="avp")
                    for j in range(NJ):
                        sp = psS.tile([P, IC], F32, tag="sp")
                        for half in range(2):
                            nc.tensor.matmul(
                                sp[:, half * MMN:(half + 1) * MMN],
                                qk_sb[2 + h // 2][hp, j * P:(j + 1) * P],
                                qk_sb[h // 2][hp, i * IC + half * MMN:
                                              i * IC + (half + 1) * MMN],
                                start=True, stop=False)
                        for half in range(2):
                            hs = slice(half * MMN, (half + 1) * MMN)
                            nc.tensor.matmul(
                                sp[:, hs], id_sb[:],
                                cb_ch[:, j * IC + half * MMN:
                                      j * IC + (half + 1) * MMN],
                                start=False, stop=True)
                        et = e_pool.tile([P, IC], BF16, tag="et")
                        nc.scalar.activation(et[:], sp[:], AF.Exp)
                        for half in range(2):
                            hs = slice(half * MMN, (half + 1) * MMN)
                            nc.tensor.matmul(avp[:, hs],
                                             v_sb[j][:, h * 65:(h + 1) * 65],
                                             et[:, hs], start=(j == 0),
                                             stop=(j == NJ - 1))
                    # normalize: values/denominator -> attn_sb
                    rt = r_pool.tile([1, IC], F32R, tag="rt")
                    with nc.allow_low_precision(reason="f32r denom recip"):
                        nc.vector.reciprocal(rt[:], avp[64:65, :])
                    rbs = r_pool.tile([HS, IC], F32, tag="rbs")
                    nc.gpsimd.partition_broadcast(rbs[:], rt[:].bitcast(F32))
                    nc.vector.tensor_mul(
                        attn_sb[h // 2][hp.start:hp.start + HS, isl],
                        avp[0:HS, :], rbs[:])

        # ================= Phase C: output projection =================
        with tc.tile_pool(name="ow_pool", bufs=1) as ow_pool, \
             tc.tile_pool(name="y_pool", bufs=3) as y_pool, \
             tc.tile_pool(name="psY", bufs=3, space="PSUM") as psY:
            ow_sb = [ow_pool.tile([P, H], F32R, tag=f"ow{k}", name=f"ow{k}")
                     for k in range(2)]
            for k in range(2):
                nc.sync.dma_start(ow_sb[k][:], ow[k * P:(k + 1) * P, :])
            for m in range(H // P):
                yt = y_pool.tile([P, L], F32, tag="yt")
                for n in range(NI):
                    ps = psY.tile([P, IC], F32, tag="yp")
                    for half in range(2):
                        hs = slice(half * MMN, (half + 1) * MMN)
                        nsl = slice(n * IC + half * MMN,
                                    n * IC + (half + 1) * MMN)
                        for k in range(2):
                            nc.tensor.matmul(
                                ps[:, hs], ow_sb[k][:, m * P:(m + 1) * P],
                                attn_sb[k][:, nsl],
                                start=(k == 0), stop=(k == 1))
                    if (m + n) % 2 == 0:
                        nc.vector.tensor_copy(yt[:, n * IC:(n + 1) * IC],
                                              ps[:])
                    else:
                        nc.scalar.activation(yt[:, n * IC:(n + 1) * IC],
                                             ps[:], AF.Copy)
                    eng = nc.sync if n % 2 == 0 else nc.scalar
                    eng.dma_start(
                        yT[m * P:(m + 1) * P, n * IC:(n + 1) * IC],
                        yt[:, n * IC:(n + 1) * IC])

    nc.compile()
    return nc


def _alibi_slopes():
    n = NH // 2
    start = 2.0 ** (-(2.0 ** (-(np.log2(n) - 3.0))))
    s = np.array([start * start ** i for i in range(n)], dtype=np.float32)
    return np.concatenate([s, np.zeros(n, dtype=np.float32)])


def _build_in_maps(x, adj, weights, in_bias, gamma, out_w):
    from concurrent.futures import ThreadPoolExecutor
    slopes = _alibi_slopes()
    ar = np.arange(L, dtype=np.float32)
    dist = -np.abs(ar[None, :] - ar[:, None])
    ones = np.ones((1, MMN), dtype=np.float32)
    eye = np.eye(P, dtype=np.float32)
    ident = eye.astype(np.float16)
    adjT_by_b = [np.ascontiguousarray(adj[b, 0].T) for b in range(B)]
    xT_by_b = [np.ascontiguousarray(x[b].T) for b in range(B)]

    def _make_cb(core):
        b, g = divmod(core, 4)
        heads = [2 * g, 2 * g + 1, 8 + 2 * g, 9 + 2 * g]
        cbm = np.empty((L, NHL * L), dtype=np.float16)
        for hl, hh in enumerate(heads):
            t = gamma[0, hh, 0, 0] * adjT_by_b[b]
            if slopes[hh] != 0.0:
                t = t + slopes[hh] * dist
            cbm[:, hl * L:(hl + 1) * L] = t
        return cbm

    with ThreadPoolExecutor(max_workers=8) as ex:
        cb_by_core = list(ex.map(_make_cb, range(8)))

    in_maps = []
    for core in range(8):
        b, g = divmod(core, 4)
        heads = [2 * g, 2 * g + 1, 8 + 2 * g, 9 + 2 * g]
        xTb = xT_by_b[b]
        cbm = cb_by_core[core]

        qcols = np.concatenate([np.arange(192 * h, 192 * h + 64)
                                for h in heads])
        kcols = qcols + 64
        vcols = qcols + 128
        wqk = np.ascontiguousarray(weights[:, np.concatenate([qcols, kcols])])
        qkb = in_bias[0, 0, np.concatenate([qcols, kcols])].reshape(1, -1)
        wv = np.zeros((H, NHL * 65), dtype=np.float32)
        vbr = np.zeros((1, NHL * 65), dtype=np.float32)
        for hl in range(NHL):
            wv[:, 65 * hl:65 * hl + 64] = weights[:, vcols[64 * hl:64 * hl + 64]]
            vbr[0, 65 * hl:65 * hl + 64] = in_bias[0, 0, vcols[64 * hl:64 * hl + 64]]
            vbr[0, 65 * hl + 64] = 1.0
        owm = np.ascontiguousarray(
            out_w[np.concatenate([np.arange(64 * h, 64 * h + 64)
                                  for h in heads]), :])
        in_maps.append({
            "xT": xTb, "wqk": wqk, "wv": wv,
            "qkb": np.ascontiguousarray(qkb, dtype=np.float32),
            "vb": vbr, "ones": ones, "cb": cbm, "ident": ident,
            "ow": owm,
        })
    return in_maps


def kernel(x, adj, weights, in_bias, out_w, out_bias, gamma):
    x = np.asarray(x, dtype=np.float32)
    adj = np.asarray(adj, dtype=np.float32)
    weights = np.asarray(weights, dtype=np.float32)
    in_bias = np.asarray(in_bias, dtype=np.float32)
    out_w = np.asarray(out_w, dtype=np.float32)
    out_bias = np.asarray(out_bias, dtype=np.float32)
    gamma = np.asarray(gamma, dtype=np.float32)

    with_qk_bias = bool(np.any(in_bias[0, 0, :]))
    key = f"nc_{with_qk_bias}"
    if key not in _cache:
        _cache[key] = _build_program(with_qk_bias)
    nc = _cache[key]

    in_maps = _build_in_maps(x, adj, weights, in_bias, gamma, out_w)
    if not with_qk_bias:
        for m in in_maps:
            m.pop("qkb")
    res = bass_utils.run_bass_kernel_spmd(nc, in_maps, core_ids=list(range(8)),
                                          **RUN_KWARGS)
    _cache["last_result"] = res

    out = np.empty((B, L, H), dtype=np.float32)
    for b in range(B):
        acc = res.results[4 * b]["yT"].astype(np.float64)
        for g in range(1, 4):
            acc += res.results[4 * b + g]["yT"]
        out[b] = acc.T + out_bias[0, 0][None, :]
    return out
